# revision 30
# baseline (speedup 1.0000x reference)
"""Self-contained Trainium2 (Bass/Tile) kernel for nn_DirectPredictionGNN.

Generated from work/gnn.py — do not edit directly."""
"""DirectPredictionGNN on 8 Trainium2 NeuronCores (Bass/Tile).

Strategy (graph-partitioned pull-based GCN):
  - Nodes sharded contiguously across 8 cores; per-core relabeling by in-degree
    (output is permutation-invariant: global mean pool).
  - Per layer: each core computes hhat = dinv * (h @ W_conv) for its shard,
    AllGather -> full table in HBM, then indirect-DMA gathers hhat[src] for its
    in-edges (slots), segment-sums them per dst via one-hot matmuls into PSUM
    (slots sorted by dst window; one-hot built on DVE from iota == nrel),
    then LayerNorm/ReLU/residual epilogue per 128-node window.
  - Self-loop term folded into the epilogue (hhat tile re-read sequentially).
  - Head: partial mean-pool per core, AllReduce, tiny replicated MLP.

The program is SPMD-uniform: all per-core variation lives in input tensors
(slot src indices, slot->node-within-window values, dinv, xT). Tile structure
(windows, tiles-per-window) is shared across cores (max over cores, padded).
"""
import math
from dataclasses import dataclass, field

import numpy as np

from concourse import bass, mybir, bacc, tile

F32 = mybir.dt.float32
EPS = 1e-5


@dataclass
class Cfg:
    N: int = 100000
    E: int = 1600000
    F_IN: int = 32
    H: int = 128
    OUT: int = 200
    L: int = 4
    NC: int = 8
    CH: int = 2048          # max slots per gather call
    MAXW_RANGE: int = 3     # max windows per range
    msg_dt: object = mybir.dt.bfloat16  # hhat exchange table + gathered msgs
    msgs_bufs: int = 3

    @property
    def NLOC(self):
        assert self.N % self.NC == 0
        return self.N // self.NC

    @property
    def NLP(self):
        return ((self.NLOC + 127) // 128) * 128

    @property
    def W(self):
        return self.NLP // 128

    @property
    def NTAB(self):
        return self.NLP * self.NC


@dataclass
class Meta:
    T: object = None       # Tg [Wn, G] tiles per (window, group)
    S: int = 0
    chunks: list = field(default_factory=list)  # window ranges [(w0, w1)]
    CR: int = 0
    Tg: object = None
    Bg: object = None
    Sg: object = None
    G: int = 4
    NLP2: int = 0


def host_preprocess(cfg: Cfg, x, edge_index):
    """Return (per-core input dict pieces, meta). 4 source-stripe groups."""
    N, NC, NLOC, NLP = cfg.N, cfg.NC, cfg.NLOC, cfg.NLP
    G = 4
    NLP2 = 2 * NLP                # table-stripe rows (2 cores per stripe)
    src = edge_index[0].astype(np.int64)
    dst = edge_index[1].astype(np.int64)
    deg = np.bincount(dst, minlength=N).astype(np.float32) + 1.0  # + self loop
    dinv = (1.0 / np.sqrt(deg)).astype(np.float32)

    core_of = dst // NLOC
    CR = NLP // G                 # (unused in stripe layout; kept for meta)
    rank = np.empty(N, dtype=np.int64)
    dinv_new = np.zeros((NC, NLP), dtype=np.float32)
    xT_new = np.zeros((NC, cfg.F_IN, NLP), dtype=np.float32)
    for c in range(NC):
        lo, hi = c * NLOC, (c + 1) * NLOC
        degc = deg[lo:hi]
        order = np.argsort(-degc, kind="stable")
        r = np.empty(NLOC, dtype=np.int64)
        r[order] = np.arange(NLOC)
        rank[lo:hi] = r
        dinv_new[c, :NLOC] = dinv[lo:hi][order]
        xT_new[c, :, :NLOC] = x[lo:hi][order].T
    # table row of node v: core-major (stripe g = 2 cores)
    g_new = (np.arange(N) // NLOC) * NLP + rank
    s_grp_all = g_new // NLP2
    s_loc_all = g_new - s_grp_all * NLP2

    Wn = cfg.W
    cnt = np.zeros((NC, Wn, G), dtype=np.int64)
    per_core = []
    for c in range(NC):
        m = core_of == c
        s_g = s_loc_all[src[m]]
        grp = s_grp_all[src[m]]
        d_l = rank[dst[m]]
        o = np.lexsort((grp, d_l // 128))   # sort by window, then group
        s_g, d_l, grp = s_g[o], d_l[o], grp[o]
        w_of = d_l // 128
        np.add.at(cnt[c], (w_of, grp), 1)
        per_core.append((s_g, d_l, grp, w_of))

    # tiles per (window, group), shared across cores
    Tg = np.ceil(cnt.max(axis=0) / 128.0).astype(np.int64)   # [Wn, G]
    Sg = (Tg.sum(axis=0) * 128).astype(np.int64)             # per-group slots
    # per-group slot base of each window
    Bg = np.zeros((Wn + 1, G), dtype=np.int64)
    Bg[1:] = np.cumsum(Tg * 128, axis=0)

    srcs16 = [np.zeros((NC, int(Sg[g])), dtype=np.int16) for g in range(G)]
    nrel = [np.full((NC, int(Sg[g])), -1.0, dtype=np.float32) for g in range(G)]
    for c in range(NC):
        s_g, d_l, grp, w_of = per_core[c]
        # position within (window, group) run
        key = w_of * G + grp
        starts_flat = np.concatenate([[0], np.cumsum(cnt[c].reshape(-1))[:-1]])
        pos_in_run = np.arange(len(d_l)) - starts_flat[key]
        for g in range(G):
            m = grp == g
            slot = Bg[w_of[m], g] + pos_in_run[m]
            srcs16[g][c, slot] = s_g[m].astype(np.int16)
            nrel[g][c, slot] = (d_l[m] - 128 * w_of[m]).astype(np.float32)

    # window ranges (chunks): whole windows, <= CH slots per group, <= 3 windows
    ranges = []
    w = 0
    while w < Wn:
        w2 = w + 1
        while (w2 < Wn and w2 - w < cfg.MAXW_RANGE
               and all((Tg[w:w2 + 1, g].sum() * 128) <= cfg.CH for g in range(G))):
            w2 += 1
        ranges.append((w, w2))
        w = w2

    meta = Meta(T=Tg, S=int(Sg.sum()), chunks=ranges)
    meta.Tg = Tg
    meta.Bg = Bg
    meta.Sg = Sg
    meta.G = G
    meta.NLP2 = NLP2
    meta.CR = CR

    # device layouts
    # idx: per group [32*(g+1), Sg/16] int16, data replicated in all 16-bands
    srcs_dev = []
    for g in range(G):
        base = srcs16[g].reshape(NC, int(Sg[g]) // 16, 16).transpose(0, 2, 1)
        rep = np.tile(base, (1, 2 * (g + 1), 1))
        srcs_dev.append(np.ascontiguousarray(rep))
    BF = mybir.dt.np(mybir.dt.bfloat16)
    nrel_dev = [np.ascontiguousarray(
        nrel[g].reshape(NC, int(Sg[g]) // 128, 128).transpose(0, 2, 1))
        for g in range(G)]
    dinv_dev = np.ascontiguousarray(dinv_new.reshape(NC, Wn, 128).transpose(0, 2, 1))
    return dict(srcs_dev=srcs_dev, nrel_dev=nrel_dev, dinv_dev=dinv_dev,
                xT=xT_new.astype(BF)), meta


def prep_weights(cfg: Cfg, wts):
    """Host-side weight layouts (shared across cores)."""
    H, L = cfg.H, cfg.L
    BF = mybir.dt.np(mybir.dt.bfloat16)
    rep = lambda v: np.tile(np.asarray(v, np.float32).reshape(1, -1), (128, 1))
    d = {
        "W_emb": np.asarray(wts["W_emb"], np.float32).astype(BF),
        "W_conv": np.asarray(wts["W_conv"], np.float32).astype(BF),
        "W_res0": np.asarray(wts["W_res"][0], np.float32).astype(BF),
        "b_emb_col": np.asarray(wts["b_emb"], np.float32).reshape(H, 1),
        "b_conv_rep": np.stack([rep(wts["b_conv"][i]) for i in range(L)]),
        "g_rep": np.stack([rep(wts["ln_g"][i]) for i in range(L)]),
        "bln_rep": np.stack([rep(wts["ln_b"][i]) for i in range(L)]),
        "b_res_rep": rep(wts["b_res"][0]),
        "W_fc1": np.asarray(wts["W_fc1"], np.float32),
        "b_fc1": np.asarray(wts["b_fc1"], np.float32).reshape(1, H),
        "fcn_g": np.asarray(wts["fcn_g"], np.float32).reshape(1, H),
        "fcn_b": np.asarray(wts["fcn_b"], np.float32).reshape(1, H),
        "W_fc2": np.asarray(wts["W_fc2"], np.float32),
        "b_fc2": np.asarray(wts["b_fc2"], np.float32).reshape(1, cfg.OUT),
        "iota": np.tile(np.arange(128, dtype=np.float32).reshape(1, 128), (128, 1)),
        "ident": np.eye(128, dtype=np.float32),
        "ident_bf": np.eye(128, dtype=np.float32).astype(BF),
        "ones_col": np.ones((128, 1), np.float32),
    }
    return d


def build_program(cfg: Cfg, meta: Meta):
    N, H, L, NC = cfg.N, cfg.H, cfg.L, cfg.NC
    NLOC, NLP, Wn, NTAB = cfg.NLOC, cfg.NLP, cfg.W, cfg.NTAB
    T, S, chunks = meta.T, meta.S, meta.chunks
    MDT = cfg.msg_dt
    msz = mybir.dt.size(MDT)

    nc = bacc.Bacc("TRN2", target_bir_lowering=False, debug=False,
                   enable_asserts=False, num_devices=NC, num_swdge_queues=4)

    # ---- I/O tensors ----
    G, Tg, Bg, Sg, NLP2 = meta.G, meta.Tg, meta.Bg, meta.Sg, meta.NLP2
    CR = meta.CR
    BF16 = mybir.dt.bfloat16
    t_srcs = [nc.dram_tensor(f"srcs{g}", [32 * (g + 1), int(Sg[g]) // 16],
                             mybir.dt.int16, kind="ExternalInput") for g in range(G)]
    t_nrel = [nc.dram_tensor(f"nrel{g}", [128, int(Sg[g]) // 128], F32,
                             kind="ExternalInput") for g in range(G)]
    t_dinv = nc.dram_tensor("dinv", [128, Wn], F32, kind="ExternalInput")
    t_xT = nc.dram_tensor("xT", [cfg.F_IN, NLP], BF16, kind="ExternalInput")
    wt = {}
    for name, shape, wdt in [
        ("W_emb", [cfg.F_IN, H], BF16), ("W_conv", [L, H, H], BF16),
        ("W_res0", [H, H], BF16),
        ("b_emb_col", [H, 1], F32), ("b_conv_rep", [L, 128, H], F32),
        ("g_rep", [L, 128, H], F32),
        ("bln_rep", [L, 128, H], F32), ("b_res_rep", [128, H], F32),
        ("W_fc1", [H, H], F32), ("b_fc1", [1, H], F32), ("fcn_g", [1, H], F32),
        ("fcn_b", [1, H], F32),
        ("W_fc2", [H, cfg.OUT], F32), ("b_fc2", [1, cfg.OUT], F32),
        ("iota", [128, 128], F32), ("ident", [128, 128], F32),
        ("ident_bf", [128, 128], BF16), ("ones_col", [128, 1], F32),
    ]:
        wt[name] = nc.dram_tensor(name, shape, wdt, kind="ExternalInput")
    t_y = nc.dram_tensor("y", [1, cfg.OUT], F32, kind="ExternalOutput")
    t_hd = nc.dram_tensor("hdump", [L + 1, NLP, H], F32, kind="ExternalOutput") \
        if getattr(cfg, "debug_dump", False) else None
    t_ad = nc.dram_tensor("adump", [NLP, H], F32, kind="ExternalOutput") \
        if getattr(cfg, "debug_dump", False) else None
    t_gd = nc.dram_tensor("gdump", [NLP, H], F32, kind="ExternalOutput") \
        if getattr(cfg, "debug_dump", False) else None

    # ---- internal DRAM ----
    # one-hot spill: built in layer 0, replayed by DMA in layers 1..L-1
    t_ohd = [nc.dram_tensor(f"ohd{g}", [128, int(Sg[g])], MDT, kind="Internal")
             for g in range(G)]
    agin = nc.dram_tensor("agin", [NLP, H], MDT, kind="Internal")
    hfull = nc.dram_tensor("hfull", [NTAB, H], MDT, kind="Internal", addr_space="Shared")
    res_dram = nc.dram_tensor("res_dram", [NLP, H], F32, kind="Internal")
    ar_in = nc.dram_tensor("ar_in", [1, H], F32, kind="Internal")
    ar_out = nc.dram_tensor("ar_out", [1, H], F32, kind="Internal", addr_space="Shared")
    ztmp = nc.dram_tensor("ztmp", [1, H], F32, kind="Internal")

    # per (window, group): number of matmul-tiles and ordering metadata.
    # Window w total tiles = sum_g Tg[w, g]; matmuls for a window are emitted
    # group-ascending; start on the first (g with Tg>0), stop on the last.
    Twin = Tg.sum(axis=1)          # total tiles per window
    dinv_v = t_dinv[:]

    with tile.TileContext(nc) as tc:
        with tc.tile_pool(name="persist", bufs=1) as pp, \
             tc.tile_pool(name="chunk", bufs=cfg.msgs_bufs) as cp, \
             tc.tile_pool(name="oh", bufs=2) as ohp, \
             tc.tile_pool(name="ep", bufs=3) as ep, \
             tc.tile_pool(name="pa", bufs=3) as pa, \
             tc.tile_pool(name="small", bufs=4) as sp, \
             tc.tile_pool(name="ptrans", bufs=2, space="PSUM") as ptrans, \
             tc.tile_pool(name="pA", bufs=2, space="PSUM") as pA, \
             tc.tile_pool(name="pacc", bufs=3, space="PSUM") as pacc:

            # ---- persistent SBUF loads ----
            h_sb = pp.tile([128, NLP], F32, tag="h")
            hhat_sb = pp.tile([128, NLP], MDT, tag="hhat_sb")
            dinv_sb = pp.tile([128, Wn], F32, tag="dinv")
            nc.sync.dma_start(dinv_sb[:], dinv_v)
            eps_col = pp.tile([128, 1], F32, tag="eps")
            nc.vector.memset(eps_col[:], EPS)
            zero_col = pp.tile([128, 1], F32, tag="zero")
            nc.vector.memset(zero_col[:], 0.0)
            sb = {}
            for name in ["W_emb", "W_conv", "W_res0", "b_emb_col", "b_conv_rep",
                         "g_rep", "bln_rep", "b_res_rep", "W_fc1", "b_fc1",
                         "fcn_g", "fcn_b", "W_fc2", "b_fc2", "iota", "ident",
                         "ident_bf", "ones_col"]:
            # flatten leading dims into free axis
                tsr = wt[name]
                shp = tsr.shape
                tdt = tsr.dtype
                if len(shp) == 3:
                    tl = pp.tile([shp[1], shp[0] * shp[2]], tdt, tag=name)
                    nc.sync.dma_start(
                        tl[:].rearrange("p (l f) -> p l f", l=shp[0]),
                        tsr[:].rearrange("l p f -> p l f"))
                    sb[name] = tl
                else:
                    tl = pp.tile(list(shp), tdt, tag=name)
                    nc.sync.dma_start(tl[:], tsr[:])
                    sb[name] = tl

            def wslice(name, i):
                tsr = wt[name]
                f = tsr.shape[2]
                return sb[name][:, i * f:(i + 1) * f]

            # ================= per-layer =================
            def dump_h(slot):
                if t_hd is None:
                    return
                for t in range(Wn):
                    nc.sync.dma_start(
                        t_hd[slot, t * 128:(t + 1) * 128, :],
                        h_sb[:, t * 128:(t + 1) * 128])
            def phase_a_window(i, t):
                """hhat production for layer i, window t (+ h for layer 0;
                + residual for layer 2). Writes hhat_sb[:, ns] and agin."""
                ns = slice(t * 128, (t + 1) * 128)
                if i == 0:
                    xT_t = pa.tile([cfg.F_IN, 128], mybir.dt.bfloat16, tag="xT")
                    nc.sync.dma_start(xT_t[:], t_xT[:, ns])
                    hT_ps = ptrans.tile([128, 128], F32, space="PSUM", tag="tps")
                    nc.tensor.matmul(hT_ps[:], lhsT=sb["W_emb"][:],
                                     rhs=xT_t[:], start=True, stop=True)
                    hT_f32 = pa.tile([128, 128], F32, tag="hT32")
                    nc.vector.tensor_tensor(
                        out=hT_f32[:], in0=hT_ps[:],
                        in1=sb["b_emb_col"][:, 0:1].to_broadcast([128, 128]),
                        op=mybir.AluOpType.add)
                    h_ps = pA.tile([128, 128], F32, space="PSUM", tag="hhps")
                    nc.tensor.transpose(h_ps[:], hT_f32[:], sb["ident"][:])
                    nc.vector.tensor_copy(h_sb[:, ns], h_ps[:])
                    hT_sb = pa.tile([128, 128], mybir.dt.bfloat16, tag="hT")
                    nc.vector.tensor_copy(hT_sb[:], hT_f32[:])
                else:
                    hT_ps = ptrans.tile([128, 128], F32, space="PSUM", tag="tps")
                    nc.tensor.transpose(hT_ps[:], h_sb[:, ns], sb["ident"][:])
                    hT_sb = pa.tile([128, 128], mybir.dt.bfloat16, tag="hT")
                    nc.vector.tensor_copy(hT_sb[:], hT_ps[:])
                hh_ps = pA.tile([128, 128], F32, space="PSUM", tag="hhps")
                nc.tensor.matmul(hh_ps[:], lhsT=hT_sb[:], rhs=wslice("W_conv", i),
                                 start=True, stop=True)
                nc.scalar.mul(hhat_sb[:, ns], hh_ps[:], dinv_sb[:, t:t + 1])
                nc.sync.dma_start(agin[ns, :], hhat_sb[:, ns])
                if i == 2:
                    r_ps = pA.tile([128, 128], F32, space="PSUM", tag="hhps")
                    nc.tensor.matmul(r_ps[:], lhsT=hT_sb[:], rhs=sb["W_res0"][:],
                                     start=True, stop=True)
                    r_sb = pa.tile([128, 128], F32, tag="rsb")
                    nc.vector.tensor_tensor(out=r_sb[:], in0=r_ps[:],
                                            in1=sb["b_res_rep"][:],
                                            op=mybir.AluOpType.add)
                    nc.sync.dma_start(res_dram[ns, :], r_sb[:])

            def do_allgather():
                nc.gpsimd.collective_compute(
                    "AllGather", mybir.AluOpType.bypass,
                    ins=[agin[:]], outs=[hfull[:]],
                    replica_groups=[list(range(NC))])

            # layer 0 embedding + hhat, then first exchange
            for t in range(Wn):
                phase_a_window(0, t)
            dump_h(0)
            do_allgather()
            if t_gd is not None:
                nc.sync.dma_start(t_gd[:], agin[:])

            for i in range(L):
                # ---- phase C: gather (4 groups / 4 SWDGE queues) + scatter;
                # phase A of layer i+1 is interleaved per finished window.
                for (w0, w1) in chunks:
                    # gather calls: one per group over this window range
                    mt = {}
                    for g in range(G):
                        nsl = int(Tg[w0:w1, g].sum()) * 128
                        if nsl == 0:
                            continue
                        nj = nsl // 128
                        j16 = int(Bg[w0, g]) // 16
                        idx_t = cp.tile([32 * (g + 1), nsl // 16],
                                        mybir.dt.int16, tag=f"idx{g}",
                                        name=f"idx_{i}_{w0}_{g}")
                        nc.sync.dma_start(
                            idx_t[:], t_srcs[g][:, j16:j16 + nsl // 16])
                        tb = int(Bg[w0, g]) // 128
                        msgs = cp.tile([128, nj * H], MDT,
                                       tag=f"msgs{g}", name=f"msgs_{i}_{w0}_{g}")
                        nc.gpsimd.dma_gather(
                            out_ap=msgs[:].rearrange("p (j h) -> p j h", h=H),
                            in_ap=hfull[g * NLP2:(g + 1) * NLP2, :],
                            idxs_ap=idx_t[:],
                            num_idxs=nsl, num_idxs_reg=nsl,
                            elem_size=H,
                            queue_num=g,
                            single_packet=False,
                        )
                        # one-hots: DVE-built in layer 0 (then spilled),
                        # DMA-replayed from DRAM in later layers
                        oh_all = ohp.tile([128, (cfg.CH // 128) * 128], MDT,
                                          tag=f"oha{g}", name=f"oha_{i}_{w0}_{g}")
                        if i == 0:
                            nrel_t = cp.tile([128, nj], F32, tag=f"nrel{g}",
                                             name=f"nrel_{i}_{w0}_{g}")
                            nc.sync.dma_start(
                                nrel_t[:], t_nrel[g][:, tb:tb + nj])
                            for jb in range(0, nj, 8):
                                B = min(8, nj - jb)
                                nc.vector.tensor_tensor(
                                    out=oh_all[:, jb * 128:(jb + B) * 128]
                                    .rearrange("p (b n) -> p b n", b=B),
                                    in0=sb["iota"][:].rearrange(
                                        "p (o n) -> p o n", o=1
                                    ).to_broadcast([128, B, 128]),
                                    in1=nrel_t[:, jb:jb + B].rearrange(
                                        "p (j o) -> p j o", o=1
                                    ).to_broadcast([128, B, 128]),
                                    op=mybir.AluOpType.is_equal)
                            nc.sync.dma_start(
                                t_ohd[g][:, tb * 128:(tb + nj) * 128],
                                oh_all[:, :nj * 128])
                        else:
                            nc.sync.dma_start(
                                oh_all[:, :nj * 128],
                                t_ohd[g][:, tb * 128:(tb + nj) * 128])
                        mt[g] = (msgs, oh_all)
                    # scatter: per window, per group, per tile
                    for w in range(w0, w1):
                        if int(Twin[w]) == 0:
                            _epilogue(nc, cfg, sb, ep, sp, h_sb, dinv_sb,
                                      hhat_sb, res_dram, None, w, i, wslice,
                                      eps_col, zero_col, t_ad)
                            if i + 1 < L:
                                phase_a_window(i + 1, w)
                            continue
                        acc = pacc.tile([128, H], F32, space="PSUM",
                                        tag="acc", name=f"acc_{i}_{w}")
                        k = 0
                        ktot = int(Twin[w])
                        for g in range(G):
                            if g not in mt:
                                continue
                            msgs, oh_all = mt[g]
                            jb0 = int(Bg[w, g] - Bg[w0, g]) // 128
                            for t in range(int(Tg[w, g])):
                                j = jb0 + t
                                nc.tensor.matmul(
                                    acc[:], lhsT=oh_all[:, j * 128:(j + 1) * 128],
                                    rhs=msgs[:, j * H:(j + 1) * H],
                                    start=(k == 0), stop=(k == ktot - 1))
                                k += 1
                        _epilogue(nc, cfg, sb, ep, sp, h_sb, dinv_sb, hhat_sb,
                                  res_dram, acc, w, i, wslice, eps_col,
                                  zero_col, t_ad)
                        if i + 1 < L:
                            phase_a_window(i + 1, w)
                if i + 1 < L:
                    dump_h(i + 1)
                    do_allgather()
            # ================= head =================
            dump_h(L)
            pool_ps = pacc.tile([1, H], F32, space="PSUM", tag="acc")
            nfull = NLOC // 128
            rem = NLOC - nfull * 128
            ntile_tot = nfull + (1 if rem else 0)
            k = 0
            for t in range(nfull):
                nc.tensor.matmul(pool_ps[:], lhsT=sb["ones_col"][:, 0:1],
                                 rhs=h_sb[:, t * 128:(t + 1) * 128],
                                 start=(k == 0), stop=(k == ntile_tot - 1))
                k += 1
            if rem:
                nc.tensor.matmul(pool_ps[:], lhsT=sb["ones_col"][0:rem, 0:1],
                                 rhs=h_sb[0:rem, nfull * 128:nfull * 128 + 128],
                                 start=(k == 0), stop=True)
            pool_sb = sp.tile([1, H], F32, tag="rowA")
            nc.vector.tensor_scalar(out=pool_sb[:], in0=pool_ps[:],
                                    scalar1=1.0 / N, scalar2=None,
                                    op0=mybir.AluOpType.mult)
            nc.sync.dma_start(ar_in[:], pool_sb[:])
            nc.gpsimd.collective_compute(
                "AllReduce", mybir.AluOpType.add,
                ins=[ar_in[:]], outs=[ar_out[:]],
                replica_groups=[list(range(NC))])
            pooled_col = sp.tile([128, 1], F32, tag="colA")
            nc.sync.dma_start(pooled_col[:], ar_out[:].rearrange("o f -> f o"))
            z_ps = pA.tile([1, H], F32, space="PSUM", tag="hhps")
            nc.tensor.matmul(z_ps[:], lhsT=pooled_col[:], rhs=sb["W_fc1"][:],
                             start=True, stop=True)
            z1 = sp.tile([1, H], F32, tag="rowB")
            nc.vector.tensor_tensor(out=z1[:], in0=z_ps[:], in1=sb["b_fc1"][:],
                                    op=mybir.AluOpType.add)
            # LN over [1, H]
            mus = sp.tile([1, 1], F32, tag="s1")
            nc.vector.tensor_reduce(out=mus[:], in_=z1[:], axis=mybir.AxisListType.X,
                                    op=mybir.AluOpType.add)
            mu = sp.tile([1, 1], F32, tag="s2")
            nc.vector.tensor_scalar(out=mu[:], in0=mus[:], scalar1=1.0 / H,
                                    scalar2=None, op0=mybir.AluOpType.mult)
            xc = sp.tile([1, H], F32, tag="rowC")
            nc.vector.tensor_scalar(out=xc[:], in0=z1[:], scalar1=mu[:, 0:1],
                                    scalar2=None, op0=mybir.AluOpType.subtract)
            sqt = sp.tile([1, H], F32, tag="rowD")
            vars_ = sp.tile([1, 1], F32, tag="s3")
            nc.scalar.activation(sqt[:], xc[:], mybir.ActivationFunctionType.Square,
                                 bias=zero_col[0:1, 0:1], accum_out=vars_[:, 0:1])
            std = sp.tile([1, 1], F32, tag="s4")
            nc.scalar.activation(std[:], vars_[:],
                                 mybir.ActivationFunctionType.Sqrt,
                                 bias=eps_col[0:1, 0:1], scale=1.0 / H)
            rstd = sp.tile([1, 1], F32, tag="s5")
            nc.vector.reciprocal(rstd[:], std[:])
            nc.vector.tensor_scalar(out=xc[:], in0=xc[:], scalar1=rstd[:, 0:1],
                                    scalar2=None, op0=mybir.AluOpType.mult)
            nc.vector.tensor_tensor(out=xc[:], in0=xc[:], in1=sb["fcn_g"][:],
                                    op=mybir.AluOpType.mult)
            nc.vector.tensor_tensor(out=xc[:], in0=xc[:], in1=sb["fcn_b"][:],
                                    op=mybir.AluOpType.add)
            nc.vector.tensor_scalar(out=xc[:], in0=xc[:], scalar1=0.0,
                                    scalar2=None, op0=mybir.AluOpType.max)
            nc.sync.dma_start(ztmp[:], xc[:])
            z_col = sp.tile([128, 1], F32, tag="colB")
            nc.sync.dma_start(z_col[:], ztmp[:].rearrange("o f -> f o"))
            y_ps = pA.tile([1, cfg.OUT], F32, space="PSUM", tag="hhps")
            nc.tensor.matmul(y_ps[:], lhsT=z_col[:], rhs=sb["W_fc2"][:],
                             start=True, stop=True)
            y_sb = sp.tile([1, cfg.OUT], F32, tag="rowE")
            nc.vector.tensor_tensor(out=y_sb[:], in0=y_ps[:], in1=sb["b_fc2"][:],
                                    op=mybir.AluOpType.add)
            nc.sync.dma_start(t_y[:], y_sb[:])

    nc.compile()
    return nc


def _epilogue(nc, cfg, sb, ep, sp, h_sb, dinv_sb, hhat_sb, res_dram, acc_ps,
              w, i, wslice, eps_col, zero_col, t_ad=None):
    H = cfg.H
    ns = slice(w * 128, (w + 1) * 128)
    t1 = ep.tile([128, H], F32, tag="t1")
    if acc_ps is not None:
        nc.vector.tensor_tensor(out=t1[:], in0=acc_ps[:], in1=hhat_sb[:, ns],
                                op=mybir.AluOpType.add)
    else:
        nc.vector.tensor_copy(t1[:], hhat_sb[:, ns])
    if i == 0 and t_ad is not None:
        nc.sync.dma_start(t_ad[ns, :], t1[:])
    # t1 = dinv * (sum + hhat_self) + b_conv
    nc.scalar.mul(t1[:], t1[:], dinv_sb[:, w:w + 1])
    nc.vector.tensor_tensor(out=t1[:], in0=t1[:], in1=wslice("b_conv_rep", i),
                            op=mybir.AluOpType.add)
    # LayerNorm
    mus = sp.tile([128, 1], F32, tag="w1")
    nc.vector.tensor_reduce(out=mus[:], in_=t1[:], axis=mybir.AxisListType.X,
                            op=mybir.AluOpType.add)
    mu = sp.tile([128, 1], F32, tag="w2")
    nc.scalar.mul(mu[:], mus[:], 1.0 / H)
    xc = ep.tile([128, H], F32, tag="xc")
    nc.vector.tensor_tensor(out=xc[:], in0=t1[:],
                            in1=mu[:, 0:1].to_broadcast([128, H]),
                            op=mybir.AluOpType.subtract)
    sqt = ep.tile([128, H], F32, tag="sqt")
    vars_ = sp.tile([128, 1], F32, tag="w3")
    nc.scalar.activation(sqt[:], xc[:], mybir.ActivationFunctionType.Square,
                         bias=zero_col[:, 0:1], accum_out=vars_[:, 0:1])
    std = sp.tile([128, 1], F32, tag="w4")
    nc.scalar.activation(std[:], vars_[:], mybir.ActivationFunctionType.Sqrt,
                         bias=eps_col[:, 0:1], scale=1.0 / H)
    rstd = sp.tile([128, 1], F32, tag="w5")
    nc.vector.reciprocal(rstd[:], std[:])
    # y = relu(xc * rstd * g + b) + residual
    nc.vector.tensor_tensor(out=xc[:], in0=xc[:], in1=wslice("g_rep", i),
                            op=mybir.AluOpType.mult)
    nc.scalar.mul(xc[:], xc[:], rstd[:, 0:1])  # ACT: xc *= rstd
    nc.vector.tensor_tensor(out=xc[:], in0=xc[:], in1=wslice("bln_rep", i),
                            op=mybir.AluOpType.add)
    relu = ep.tile([128, H], F32, tag="relu")
    nc.scalar.activation(relu[:], xc[:], mybir.ActivationFunctionType.Relu,
                         bias=zero_col[:, 0:1])
    if i == 2:
        res_r = ep.tile([128, H], F32, tag="resr")
        nc.sync.dma_start(res_r[:], res_dram[ns, :])
        nc.vector.tensor_tensor(out=h_sb[:, ns], in0=relu[:], in1=res_r[:],
                                op=mybir.AluOpType.add)
    else:
        nc.vector.tensor_tensor(out=h_sb[:, ns], in0=relu[:], in1=h_sb[:, ns],
                                op=mybir.AluOpType.add)




# ======================= axon NTFF profiling shim =======================

import contextlib
import ctypes
import sys
import types

_SO_PATH = "/opt/axon/libaxon_pjrt.so"


def _ntff_profile_via_ctypes(so_path: str):
    lib = ctypes.CDLL(so_path)
    if not hasattr(lib, "axon_start_nrt_profile"):
        return None
    lib.axon_start_nrt_profile.argtypes = [
        ctypes.POINTER(ctypes.c_int64),
        ctypes.c_size_t,
    ]
    lib.axon_start_nrt_profile.restype = ctypes.c_int64
    lib.axon_stop_nrt_profile.argtypes = [ctypes.c_char_p]
    lib.axon_stop_nrt_profile.restype = ctypes.c_int64

    @contextlib.contextmanager
    def _hook(output_dir: str, device_ids):
        import jax

        jax.devices()
        if device_ids:
            ids = (ctypes.c_int64 * len(device_ids))(*device_ids)
            rc = lib.axon_start_nrt_profile(ids, len(device_ids))
        else:
            rc = lib.axon_start_nrt_profile(None, 0)
        if rc != 0:
            raise RuntimeError(f"axon_start_nrt_profile rc={rc}")
        try:
            yield
        finally:
            n = lib.axon_stop_nrt_profile(str(output_dir).encode())
            if n < 0:
                raise RuntimeError(f"axon_stop_nrt_profile rc={n}")
            print(f"profile: {n} file(s) written to {output_dir}", file=sys.stderr)

    return _hook


def _shim_install():
    try:
        import antenv.axon_hooks  # noqa: F401

        return  # already present
    except ImportError:
        pass
    import antenv

    hook = _ntff_profile_via_ctypes(_SO_PATH)
    mod = types.ModuleType("antenv.axon_hooks")
    mod._hook = hook

    def get_axon_ntff_profile_hook():
        return mod._hook

    def set_axon_ntff_profile_hook(h):
        mod._hook = h

    mod.get_axon_ntff_profile_hook = get_axon_ntff_profile_hook
    mod.set_axon_ntff_profile_hook = set_axon_ntff_profile_hook
    sys.modules["antenv.axon_hooks"] = mod
    antenv.axon_hooks = mod


# ======================= kernel entry point =======================
_CACHE = {}
TRACE = False
LAST_EXEC_NS = None


def kernel(**inputs):
    """Full unsharded inputs -> full output [1, 200] (float32).

    Shards the graph across 8 NeuronCores internally (node partitioning with
    per-core degree relabeling; AllGather exchange of the normalized conv
    features per layer; per-core gather/scatter message passing; AllReduce
    mean-pool; replicated head).
    """
    global LAST_EXEC_NS
    from concourse import bass_utils

    cfg = Cfg()
    x = np.asarray(inputs["x"], np.float32)
    edge_index = np.asarray(inputs["edge_index"])
    assert x.shape == (cfg.N, cfg.F_IN) and edge_index.shape == (2, cfg.E)

    pc, meta = host_preprocess(cfg, x, edge_index)
    wd = prep_weights(cfg, inputs)

    key = "prog"
    if key not in _CACHE:
        _CACHE[key] = build_program(cfg, meta)
    nc = _CACHE[key]

    in_maps = []
    for c in range(cfg.NC):
        m = {"dinv": pc["dinv_dev"][c], "xT": np.ascontiguousarray(pc["xT"][c])}
        for g in range(meta.G):
            m[f"srcs{g}"] = pc["srcs_dev"][g][c]
            m[f"nrel{g}"] = pc["nrel_dev"][g][c]
        m.update(wd)
        in_maps.append(m)

    trace = TRACE
    if trace:
        try:
            _shim_install()
        except Exception:
            trace = False
    try:
        res = bass_utils.run_bass_kernel_spmd(
            nc, in_maps, core_ids=list(range(cfg.NC)), trace=trace)
    except Exception:
        if not trace:
            raise
        res = bass_utils.run_bass_kernel_spmd(
            nc, in_maps, core_ids=list(range(cfg.NC)), trace=False)
    LAST_EXEC_NS = res.exec_time_ns
    return np.asarray(res.results[0]["y"], np.float32)



# revision 32
# speedup vs baseline: 1.2607x; 1.2607x over previous
"""Self-contained Trainium2 (Bass/Tile) kernel for nn_DirectPredictionGNN.

Generated from work/gnn.py — do not edit directly."""
"""DirectPredictionGNN on 8 Trainium2 NeuronCores (Bass/Tile).

Strategy (graph-partitioned pull-based GCN):
  - Nodes sharded contiguously across 8 cores; per-core relabeling by in-degree
    (output is permutation-invariant: global mean pool).
  - Per layer: each core computes hhat = dinv * (h @ W_conv) for its shard,
    AllGather -> full table in HBM, then indirect-DMA gathers hhat[src] for its
    in-edges (slots), segment-sums them per dst via one-hot matmuls into PSUM
    (slots sorted by dst window; one-hot built on DVE from iota == nrel),
    then LayerNorm/ReLU/residual epilogue per 128-node window.
  - Self-loop term folded into the epilogue (hhat tile re-read sequentially).
  - Head: partial mean-pool per core, AllReduce, tiny replicated MLP.

The program is SPMD-uniform: all per-core variation lives in input tensors
(slot src indices, slot->node-within-window values, dinv, xT). Tile structure
(windows, tiles-per-window) is shared across cores (max over cores, padded).
"""
import math
from dataclasses import dataclass, field

import numpy as np

from concourse import bass, mybir, bacc, tile

F32 = mybir.dt.float32
EPS = 1e-5


@dataclass
class Cfg:
    N: int = 100000
    E: int = 1600000
    F_IN: int = 32
    H: int = 128
    OUT: int = 200
    L: int = 4
    NC: int = 8
    CH: int = 2048          # max slots per gather call
    MAXW_RANGE: int = 3     # max windows per range
    msg_dt: object = mybir.dt.bfloat16  # hhat exchange table + gathered msgs
    msgs_bufs: int = 3

    @property
    def NLOC(self):
        assert self.N % self.NC == 0
        return self.N // self.NC

    @property
    def NLP(self):
        return ((self.NLOC + 127) // 128) * 128

    @property
    def W(self):
        return self.NLP // 128

    @property
    def NTAB(self):
        return self.NLP * self.NC


@dataclass
class Meta:
    T: object = None       # Tg [Wn, G] tiles per (window, group)
    S: int = 0
    chunks: list = field(default_factory=list)  # window ranges [(w0, w1)]
    CR: int = 0
    Tg: object = None
    Bg: object = None
    Sg: object = None
    G: int = 4
    NLP2: int = 0


def host_preprocess(cfg: Cfg, x, edge_index):
    """Return (per-core input dict pieces, meta). 4 source-stripe groups."""
    N, NC, NLOC, NLP = cfg.N, cfg.NC, cfg.NLOC, cfg.NLP
    G = 4
    NLP2 = 2 * NLP                # table-stripe rows (2 cores per stripe)
    src = edge_index[0].astype(np.int64)
    dst = edge_index[1].astype(np.int64)
    deg = np.bincount(dst, minlength=N).astype(np.float32) + 1.0  # + self loop
    dinv = (1.0 / np.sqrt(deg)).astype(np.float32)

    core_of = dst // NLOC
    CR = NLP // G                 # (unused in stripe layout; kept for meta)
    rank = np.empty(N, dtype=np.int64)
    dinv_new = np.zeros((NC, NLP), dtype=np.float32)
    xT_new = np.zeros((NC, cfg.F_IN, NLP), dtype=np.float32)
    for c in range(NC):
        lo, hi = c * NLOC, (c + 1) * NLOC
        degc = deg[lo:hi]
        order = np.argsort(-degc, kind="stable")
        r = np.empty(NLOC, dtype=np.int64)
        r[order] = np.arange(NLOC)
        rank[lo:hi] = r
        dinv_new[c, :NLOC] = dinv[lo:hi][order]
        xT_new[c, :, :NLOC] = x[lo:hi][order].T
    # table row of node v: core-major (stripe g = 2 cores)
    g_new = (np.arange(N) // NLOC) * NLP + rank
    s_grp_all = g_new // NLP2
    s_loc_all = g_new - s_grp_all * NLP2

    Wn = cfg.W
    cnt = np.zeros((NC, Wn, G), dtype=np.int64)
    per_core = []
    for c in range(NC):
        m = core_of == c
        s_g = s_loc_all[src[m]]
        grp = s_grp_all[src[m]]
        d_l = rank[dst[m]]
        o = np.lexsort((grp, d_l // 128))   # sort by window, then group
        s_g, d_l, grp = s_g[o], d_l[o], grp[o]
        w_of = d_l // 128
        np.add.at(cnt[c], (w_of, grp), 1)
        per_core.append((s_g, d_l, grp, w_of))

    # tiles per (window, group), shared across cores
    Tg = np.ceil(cnt.max(axis=0) / 128.0).astype(np.int64)   # [Wn, G]
    Sg = (Tg.sum(axis=0) * 128).astype(np.int64)             # per-group slots
    # per-group slot base of each window
    Bg = np.zeros((Wn + 1, G), dtype=np.int64)
    Bg[1:] = np.cumsum(Tg * 128, axis=0)

    srcs16 = [np.zeros((NC, int(Sg[g])), dtype=np.int16) for g in range(G)]
    nrel = [np.full((NC, int(Sg[g])), -1.0, dtype=np.float32) for g in range(G)]
    for c in range(NC):
        s_g, d_l, grp, w_of = per_core[c]
        # position within (window, group) run
        key = w_of * G + grp
        starts_flat = np.concatenate([[0], np.cumsum(cnt[c].reshape(-1))[:-1]])
        pos_in_run = np.arange(len(d_l)) - starts_flat[key]
        for g in range(G):
            m = grp == g
            slot = Bg[w_of[m], g] + pos_in_run[m]
            srcs16[g][c, slot] = s_g[m].astype(np.int16)
            nrel[g][c, slot] = (d_l[m] - 128 * w_of[m]).astype(np.float32)

    # window ranges (chunks): whole windows, <= CH slots per group, <= 3 windows
    ranges = []
    w = 0
    while w < Wn:
        w2 = w + 1
        while (w2 < Wn and w2 - w < cfg.MAXW_RANGE
               and all((Tg[w:w2 + 1, g].sum() * 128) <= cfg.CH for g in range(G))):
            w2 += 1
        ranges.append((w, w2))
        w = w2

    meta = Meta(T=Tg, S=int(Sg.sum()), chunks=ranges)
    meta.Tg = Tg
    meta.Bg = Bg
    meta.Sg = Sg
    meta.G = G
    meta.NLP2 = NLP2
    meta.CR = CR

    # device layouts
    # idx: per group [32*(g+1), Sg/16] int16, data replicated in all 16-bands
    srcs_dev = []
    for g in range(G):
        base = srcs16[g].reshape(NC, int(Sg[g]) // 16, 16).transpose(0, 2, 1)
        rep = np.tile(base, (1, 2 * (g + 1), 1))
        srcs_dev.append(np.ascontiguousarray(rep))
    BF = mybir.dt.np(mybir.dt.bfloat16)
    nrel_dev = [np.ascontiguousarray(
        nrel[g].reshape(NC, int(Sg[g]) // 128, 128).transpose(0, 2, 1))
        for g in range(G)]
    dinv_dev = np.ascontiguousarray(dinv_new.reshape(NC, Wn, 128).transpose(0, 2, 1))
    return dict(srcs_dev=srcs_dev, nrel_dev=nrel_dev, dinv_dev=dinv_dev,
                xT=xT_new.astype(BF)), meta


def prep_weights(cfg: Cfg, wts):
    """Host-side weight layouts (shared across cores)."""
    H, L = cfg.H, cfg.L
    BF = mybir.dt.np(mybir.dt.bfloat16)
    rep = lambda v: np.tile(np.asarray(v, np.float32).reshape(1, -1), (128, 1))
    d = {
        "W_emb": np.asarray(wts["W_emb"], np.float32).astype(BF),
        "W_conv": np.asarray(wts["W_conv"], np.float32).astype(BF),
        "W_res0": np.asarray(wts["W_res"][0], np.float32).astype(BF),
        "b_emb_col": np.asarray(wts["b_emb"], np.float32).reshape(H, 1),
        "b_conv_rep": np.stack([rep(wts["b_conv"][i]) for i in range(L)]),
        "g_rep": np.stack([rep(wts["ln_g"][i]) for i in range(L)]),
        "bln_rep": np.stack([rep(wts["ln_b"][i]) for i in range(L)]),
        "b_res_rep": rep(wts["b_res"][0]),
        "W_fc1": np.asarray(wts["W_fc1"], np.float32),
        "b_fc1": np.asarray(wts["b_fc1"], np.float32).reshape(1, H),
        "fcn_g": np.asarray(wts["fcn_g"], np.float32).reshape(1, H),
        "fcn_b": np.asarray(wts["fcn_b"], np.float32).reshape(1, H),
        "W_fc2": np.asarray(wts["W_fc2"], np.float32),
        "b_fc2": np.asarray(wts["b_fc2"], np.float32).reshape(1, cfg.OUT),
        "iota": np.tile(np.arange(128, dtype=np.float32).reshape(1, 128), (128, 1)),
        "ident": np.eye(128, dtype=np.float32),
        "ident_bf": np.eye(128, dtype=np.float32).astype(BF),
        "ones_col": np.ones((128, 1), np.float32),
    }
    return d


def build_program(cfg: Cfg, meta: Meta):
    N, H, L, NC = cfg.N, cfg.H, cfg.L, cfg.NC
    NLOC, NLP, Wn, NTAB = cfg.NLOC, cfg.NLP, cfg.W, cfg.NTAB
    T, S, chunks = meta.T, meta.S, meta.chunks
    MDT = cfg.msg_dt
    msz = mybir.dt.size(MDT)

    nc = bacc.Bacc("TRN2", target_bir_lowering=False, debug=False,
                   enable_asserts=False, num_devices=NC, num_swdge_queues=4)

    # ---- I/O tensors ----
    G, Tg, Bg, Sg, NLP2 = meta.G, meta.Tg, meta.Bg, meta.Sg, meta.NLP2
    CR = meta.CR
    BF16 = mybir.dt.bfloat16
    t_srcs = [nc.dram_tensor(f"srcs{g}", [32 * (g + 1), int(Sg[g]) // 16],
                             mybir.dt.int16, kind="ExternalInput") for g in range(G)]
    t_nrel = [nc.dram_tensor(f"nrel{g}", [128, int(Sg[g]) // 128], F32,
                             kind="ExternalInput") for g in range(G)]
    t_dinv = nc.dram_tensor("dinv", [128, Wn], F32, kind="ExternalInput")
    t_xT = nc.dram_tensor("xT", [cfg.F_IN, NLP], BF16, kind="ExternalInput")
    wt = {}
    for name, shape, wdt in [
        ("W_emb", [cfg.F_IN, H], BF16), ("W_conv", [L, H, H], BF16),
        ("W_res0", [H, H], BF16),
        ("b_emb_col", [H, 1], F32), ("b_conv_rep", [L, 128, H], F32),
        ("g_rep", [L, 128, H], F32),
        ("bln_rep", [L, 128, H], F32), ("b_res_rep", [128, H], F32),
        ("W_fc1", [H, H], F32), ("b_fc1", [1, H], F32), ("fcn_g", [1, H], F32),
        ("fcn_b", [1, H], F32),
        ("W_fc2", [H, cfg.OUT], F32), ("b_fc2", [1, cfg.OUT], F32),
        ("iota", [128, 128], F32), ("ident", [128, 128], F32),
        ("ident_bf", [128, 128], BF16), ("ones_col", [128, 1], F32),
    ]:
        wt[name] = nc.dram_tensor(name, shape, wdt, kind="ExternalInput")
    t_y = nc.dram_tensor("y", [1, cfg.OUT], F32, kind="ExternalOutput")
    t_hd = nc.dram_tensor("hdump", [L + 1, NLP, H], F32, kind="ExternalOutput") \
        if getattr(cfg, "debug_dump", False) else None
    t_ad = nc.dram_tensor("adump", [NLP, H], F32, kind="ExternalOutput") \
        if getattr(cfg, "debug_dump", False) else None
    t_gd = nc.dram_tensor("gdump", [NLP, H], F32, kind="ExternalOutput") \
        if getattr(cfg, "debug_dump", False) else None

    # ---- internal DRAM ----
    agin = nc.dram_tensor("agin", [NLP, H], MDT, kind="Internal")
    hfull = nc.dram_tensor("hfull", [NTAB, H], MDT, kind="Internal", addr_space="Shared")
    res_dram = nc.dram_tensor("res_dram", [NLP, H], F32, kind="Internal")
    ar_in = nc.dram_tensor("ar_in", [1, H], F32, kind="Internal")
    ar_out = nc.dram_tensor("ar_out", [1, H], F32, kind="Internal", addr_space="Shared")
    ztmp = nc.dram_tensor("ztmp", [1, H], F32, kind="Internal")

    # per (window, group): number of matmul-tiles and ordering metadata.
    # Window w total tiles = sum_g Tg[w, g]; matmuls for a window are emitted
    # group-ascending; start on the first (g with Tg>0), stop on the last.
    Twin = Tg.sum(axis=1)          # total tiles per window
    dinv_v = t_dinv[:]

    with tile.TileContext(nc) as tc:
        with tc.tile_pool(name="persist", bufs=1) as pp, \
             tc.tile_pool(name="chunk", bufs=cfg.msgs_bufs) as cp, \
             tc.tile_pool(name="oh", bufs=2) as ohp, \
             tc.tile_pool(name="ep", bufs=3) as ep, \
             tc.tile_pool(name="pa", bufs=3) as pa, \
             tc.tile_pool(name="small", bufs=4) as sp, \
             tc.tile_pool(name="ptrans", bufs=2, space="PSUM") as ptrans, \
             tc.tile_pool(name="pA", bufs=2, space="PSUM") as pA, \
             tc.tile_pool(name="pacc", bufs=3, space="PSUM") as pacc:

            # ---- persistent SBUF loads ----
            h_sb = pp.tile([128, NLP], F32, tag="h")
            hhat_sb = pp.tile([128, NLP], MDT, tag="hhat_sb")
            dinv_sb = pp.tile([128, Wn], F32, tag="dinv")
            nc.sync.dma_start(dinv_sb[:], dinv_v)
            eps_col = pp.tile([128, 1], F32, tag="eps")
            nc.vector.memset(eps_col[:], EPS)
            zero_col = pp.tile([128, 1], F32, tag="zero")
            nc.vector.memset(zero_col[:], 0.0)
            sb = {}
            for name in ["W_emb", "W_conv", "W_res0", "b_emb_col", "b_conv_rep",
                         "g_rep", "bln_rep", "b_res_rep", "W_fc1", "b_fc1",
                         "fcn_g", "fcn_b", "W_fc2", "b_fc2", "iota", "ident",
                         "ident_bf", "ones_col"]:
            # flatten leading dims into free axis
                tsr = wt[name]
                shp = tsr.shape
                tdt = tsr.dtype
                if len(shp) == 3:
                    tl = pp.tile([shp[1], shp[0] * shp[2]], tdt, tag=name)
                    nc.sync.dma_start(
                        tl[:].rearrange("p (l f) -> p l f", l=shp[0]),
                        tsr[:].rearrange("l p f -> p l f"))
                    sb[name] = tl
                else:
                    tl = pp.tile(list(shp), tdt, tag=name)
                    nc.sync.dma_start(tl[:], tsr[:])
                    sb[name] = tl

            def wslice(name, i):
                tsr = wt[name]
                f = tsr.shape[2]
                return sb[name][:, i * f:(i + 1) * f]

            # ================= per-layer =================
            def dump_h(slot):
                if t_hd is None:
                    return
                for t in range(Wn):
                    nc.sync.dma_start(
                        t_hd[slot, t * 128:(t + 1) * 128, :],
                        h_sb[:, t * 128:(t + 1) * 128])
            def phase_a_window(i, t):
                """hhat production for layer i, window t (+ h for layer 0;
                + residual for layer 2). Writes hhat_sb[:, ns] and agin."""
                ns = slice(t * 128, (t + 1) * 128)
                if i == 0:
                    xT_t = pa.tile([cfg.F_IN, 128], mybir.dt.bfloat16, tag="xT")
                    nc.sync.dma_start(xT_t[:], t_xT[:, ns])
                    hT_ps = ptrans.tile([128, 128], F32, space="PSUM", tag="tps")
                    nc.tensor.matmul(hT_ps[:], lhsT=sb["W_emb"][:],
                                     rhs=xT_t[:], start=True, stop=True)
                    hT_f32 = pa.tile([128, 128], F32, tag="hT32")
                    nc.vector.tensor_tensor(
                        out=hT_f32[:], in0=hT_ps[:],
                        in1=sb["b_emb_col"][:, 0:1].to_broadcast([128, 128]),
                        op=mybir.AluOpType.add)
                    h_ps = pA.tile([128, 128], F32, space="PSUM", tag="hhps")
                    nc.tensor.transpose(h_ps[:], hT_f32[:], sb["ident"][:])
                    nc.vector.tensor_copy(h_sb[:, ns], h_ps[:])
                    hT_sb = pa.tile([128, 128], mybir.dt.bfloat16, tag="hT")
                    nc.vector.tensor_copy(hT_sb[:], hT_f32[:])
                else:
                    hT_ps = ptrans.tile([128, 128], F32, space="PSUM", tag="tps")
                    nc.tensor.transpose(hT_ps[:], h_sb[:, ns], sb["ident"][:])
                    hT_sb = pa.tile([128, 128], mybir.dt.bfloat16, tag="hT")
                    nc.vector.tensor_copy(hT_sb[:], hT_ps[:])
                hh_ps = pA.tile([128, 128], F32, space="PSUM", tag="hhps")
                nc.tensor.matmul(hh_ps[:], lhsT=hT_sb[:], rhs=wslice("W_conv", i),
                                 start=True, stop=True)
                nc.scalar.mul(hhat_sb[:, ns], hh_ps[:], dinv_sb[:, t:t + 1])
                nc.sync.dma_start(agin[ns, :], hhat_sb[:, ns])
                if i == 2:
                    r_ps = pA.tile([128, 128], F32, space="PSUM", tag="hhps")
                    nc.tensor.matmul(r_ps[:], lhsT=hT_sb[:], rhs=sb["W_res0"][:],
                                     start=True, stop=True)
                    r_sb = pa.tile([128, 128], F32, tag="rsb")
                    nc.vector.tensor_tensor(out=r_sb[:], in0=r_ps[:],
                                            in1=sb["b_res_rep"][:],
                                            op=mybir.AluOpType.add)
                    nc.sync.dma_start(res_dram[ns, :], r_sb[:])

            def do_allgather():
                nc.gpsimd.collective_compute(
                    "AllGather", mybir.AluOpType.bypass,
                    ins=[agin[:]], outs=[hfull[:]],
                    replica_groups=[list(range(NC))])

            # layer 0 embedding + hhat, then first exchange
            for t in range(Wn):
                phase_a_window(0, t)
            dump_h(0)
            do_allgather()
            if t_gd is not None:
                nc.sync.dma_start(t_gd[:], agin[:])

            for i in range(L):
                # ---- phase C: gather (4 groups / 4 SWDGE queues) + scatter;
                # phase A of layer i+1 is interleaved per finished window.
                for (w0, w1) in chunks:
                    # gather calls: one per group over this window range
                    mt = {}
                    for g in range(G):
                        nsl = int(Tg[w0:w1, g].sum()) * 128
                        if nsl == 0:
                            continue
                        nj = nsl // 128
                        j16 = int(Bg[w0, g]) // 16
                        idx_t = cp.tile([32 * (g + 1), nsl // 16],
                                        mybir.dt.int16, tag=f"idx{g}",
                                        name=f"idx_{i}_{w0}_{g}")
                        nc.sync.dma_start(
                            idx_t[:], t_srcs[g][:, j16:j16 + nsl // 16])
                        tb = int(Bg[w0, g]) // 128
                        msgs = cp.tile([128, nj * H], MDT,
                                       tag=f"msgs{g}", name=f"msgs_{i}_{w0}_{g}")
                        nc.gpsimd.dma_gather(
                            out_ap=msgs[:].rearrange("p (j h) -> p j h", h=H),
                            in_ap=hfull[g * NLP2:(g + 1) * NLP2, :],
                            idxs_ap=idx_t[:],
                            num_idxs=nsl, num_idxs_reg=nsl,
                            elem_size=H,
                            queue_num=g,
                            single_packet=False,
                        )
                        # one-hots: DVE-built per chunk (batched is_eq)
                        oh_all = ohp.tile([128, (cfg.CH // 128) * 128], MDT,
                                          tag=f"oha{g}", name=f"oha_{i}_{w0}_{g}")
                        nrel_t = cp.tile([128, nj], F32, tag=f"nrel{g}",
                                         name=f"nrel_{i}_{w0}_{g}")
                        nc.sync.dma_start(
                            nrel_t[:], t_nrel[g][:, tb:tb + nj])
                        for jb in range(0, nj, 8):
                            B = min(8, nj - jb)
                            nc.vector.tensor_tensor(
                                out=oh_all[:, jb * 128:(jb + B) * 128]
                                .rearrange("p (b n) -> p b n", b=B),
                                in0=sb["iota"][:].rearrange(
                                    "p (o n) -> p o n", o=1
                                ).to_broadcast([128, B, 128]),
                                in1=nrel_t[:, jb:jb + B].rearrange(
                                    "p (j o) -> p j o", o=1
                                ).to_broadcast([128, B, 128]),
                                op=mybir.AluOpType.is_equal)
                        mt[g] = (msgs, oh_all)
                    # scatter: per window, per group, per tile
                    for w in range(w0, w1):
                        if int(Twin[w]) == 0:
                            _epilogue(nc, cfg, sb, ep, sp, h_sb, dinv_sb,
                                      hhat_sb, res_dram, None, w, i, wslice,
                                      eps_col, zero_col, t_ad)
                            if i + 1 < L:
                                phase_a_window(i + 1, w)
                            continue
                        acc = pacc.tile([128, H], F32, space="PSUM",
                                        tag="acc", name=f"acc_{i}_{w}")
                        k = 0
                        ktot = int(Twin[w])
                        for g in range(G):
                            if g not in mt:
                                continue
                            msgs, oh_all = mt[g]
                            jb0 = int(Bg[w, g] - Bg[w0, g]) // 128
                            for t in range(int(Tg[w, g])):
                                j = jb0 + t
                                nc.tensor.matmul(
                                    acc[:], lhsT=oh_all[:, j * 128:(j + 1) * 128],
                                    rhs=msgs[:, j * H:(j + 1) * H],
                                    start=(k == 0), stop=(k == ktot - 1))
                                k += 1
                        _epilogue(nc, cfg, sb, ep, sp, h_sb, dinv_sb, hhat_sb,
                                  res_dram, acc, w, i, wslice, eps_col,
                                  zero_col, t_ad)
                        if i + 1 < L:
                            phase_a_window(i + 1, w)
                if i + 1 < L:
                    dump_h(i + 1)
                    do_allgather()
            # ================= head =================
            dump_h(L)
            pool_ps = pacc.tile([1, H], F32, space="PSUM", tag="acc")
            nfull = NLOC // 128
            rem = NLOC - nfull * 128
            ntile_tot = nfull + (1 if rem else 0)
            k = 0
            for t in range(nfull):
                nc.tensor.matmul(pool_ps[:], lhsT=sb["ones_col"][:, 0:1],
                                 rhs=h_sb[:, t * 128:(t + 1) * 128],
                                 start=(k == 0), stop=(k == ntile_tot - 1))
                k += 1
            if rem:
                nc.tensor.matmul(pool_ps[:], lhsT=sb["ones_col"][0:rem, 0:1],
                                 rhs=h_sb[0:rem, nfull * 128:nfull * 128 + 128],
                                 start=(k == 0), stop=True)
            pool_sb = sp.tile([1, H], F32, tag="rowA")
            nc.vector.tensor_scalar(out=pool_sb[:], in0=pool_ps[:],
                                    scalar1=1.0 / N, scalar2=None,
                                    op0=mybir.AluOpType.mult)
            nc.sync.dma_start(ar_in[:], pool_sb[:])
            nc.gpsimd.collective_compute(
                "AllReduce", mybir.AluOpType.add,
                ins=[ar_in[:]], outs=[ar_out[:]],
                replica_groups=[list(range(NC))])
            pooled_col = sp.tile([128, 1], F32, tag="colA")
            nc.sync.dma_start(pooled_col[:], ar_out[:].rearrange("o f -> f o"))
            z_ps = pA.tile([1, H], F32, space="PSUM", tag="hhps")
            nc.tensor.matmul(z_ps[:], lhsT=pooled_col[:], rhs=sb["W_fc1"][:],
                             start=True, stop=True)
            z1 = sp.tile([1, H], F32, tag="rowB")
            nc.vector.tensor_tensor(out=z1[:], in0=z_ps[:], in1=sb["b_fc1"][:],
                                    op=mybir.AluOpType.add)
            # LN over [1, H]
            mus = sp.tile([1, 1], F32, tag="s1")
            nc.vector.tensor_reduce(out=mus[:], in_=z1[:], axis=mybir.AxisListType.X,
                                    op=mybir.AluOpType.add)
            mu = sp.tile([1, 1], F32, tag="s2")
            nc.vector.tensor_scalar(out=mu[:], in0=mus[:], scalar1=1.0 / H,
                                    scalar2=None, op0=mybir.AluOpType.mult)
            xc = sp.tile([1, H], F32, tag="rowC")
            nc.vector.tensor_scalar(out=xc[:], in0=z1[:], scalar1=mu[:, 0:1],
                                    scalar2=None, op0=mybir.AluOpType.subtract)
            sqt = sp.tile([1, H], F32, tag="rowD")
            vars_ = sp.tile([1, 1], F32, tag="s3")
            nc.scalar.activation(sqt[:], xc[:], mybir.ActivationFunctionType.Square,
                                 bias=zero_col[0:1, 0:1], accum_out=vars_[:, 0:1])
            std = sp.tile([1, 1], F32, tag="s4")
            nc.scalar.activation(std[:], vars_[:],
                                 mybir.ActivationFunctionType.Sqrt,
                                 bias=eps_col[0:1, 0:1], scale=1.0 / H)
            rstd = sp.tile([1, 1], F32, tag="s5")
            nc.vector.reciprocal(rstd[:], std[:])
            nc.vector.tensor_scalar(out=xc[:], in0=xc[:], scalar1=rstd[:, 0:1],
                                    scalar2=None, op0=mybir.AluOpType.mult)
            nc.vector.tensor_tensor(out=xc[:], in0=xc[:], in1=sb["fcn_g"][:],
                                    op=mybir.AluOpType.mult)
            nc.vector.tensor_tensor(out=xc[:], in0=xc[:], in1=sb["fcn_b"][:],
                                    op=mybir.AluOpType.add)
            nc.vector.tensor_scalar(out=xc[:], in0=xc[:], scalar1=0.0,
                                    scalar2=None, op0=mybir.AluOpType.max)
            nc.sync.dma_start(ztmp[:], xc[:])
            z_col = sp.tile([128, 1], F32, tag="colB")
            nc.sync.dma_start(z_col[:], ztmp[:].rearrange("o f -> f o"))
            y_ps = pA.tile([1, cfg.OUT], F32, space="PSUM", tag="hhps")
            nc.tensor.matmul(y_ps[:], lhsT=z_col[:], rhs=sb["W_fc2"][:],
                             start=True, stop=True)
            y_sb = sp.tile([1, cfg.OUT], F32, tag="rowE")
            nc.vector.tensor_tensor(out=y_sb[:], in0=y_ps[:], in1=sb["b_fc2"][:],
                                    op=mybir.AluOpType.add)
            nc.sync.dma_start(t_y[:], y_sb[:])

    nc.compile()
    return nc


def _epilogue(nc, cfg, sb, ep, sp, h_sb, dinv_sb, hhat_sb, res_dram, acc_ps,
              w, i, wslice, eps_col, zero_col, t_ad=None):
    H = cfg.H
    ns = slice(w * 128, (w + 1) * 128)
    t1 = ep.tile([128, H], F32, tag="t1")
    if acc_ps is not None:
        nc.vector.tensor_tensor(out=t1[:], in0=acc_ps[:], in1=hhat_sb[:, ns],
                                op=mybir.AluOpType.add)
    else:
        nc.vector.tensor_copy(t1[:], hhat_sb[:, ns])
    if i == 0 and t_ad is not None:
        nc.sync.dma_start(t_ad[ns, :], t1[:])
    # t1 = dinv * (sum + hhat_self) + b_conv
    nc.scalar.mul(t1[:], t1[:], dinv_sb[:, w:w + 1])
    nc.vector.tensor_tensor(out=t1[:], in0=t1[:], in1=wslice("b_conv_rep", i),
                            op=mybir.AluOpType.add)
    # LayerNorm
    mus = sp.tile([128, 1], F32, tag="w1")
    nc.vector.tensor_reduce(out=mus[:], in_=t1[:], axis=mybir.AxisListType.X,
                            op=mybir.AluOpType.add)
    mu = sp.tile([128, 1], F32, tag="w2")
    nc.scalar.mul(mu[:], mus[:], 1.0 / H)
    xc = ep.tile([128, H], F32, tag="xc")
    nc.vector.tensor_tensor(out=xc[:], in0=t1[:],
                            in1=mu[:, 0:1].to_broadcast([128, H]),
                            op=mybir.AluOpType.subtract)
    sqt = ep.tile([128, H], F32, tag="sqt")
    vars_ = sp.tile([128, 1], F32, tag="w3")
    nc.scalar.activation(sqt[:], xc[:], mybir.ActivationFunctionType.Square,
                         bias=zero_col[:, 0:1], accum_out=vars_[:, 0:1])
    std = sp.tile([128, 1], F32, tag="w4")
    nc.scalar.activation(std[:], vars_[:], mybir.ActivationFunctionType.Sqrt,
                         bias=eps_col[:, 0:1], scale=1.0 / H)
    rstd = sp.tile([128, 1], F32, tag="w5")
    nc.vector.reciprocal(rstd[:], std[:])
    # y = relu(xc * rstd * g + b) + residual
    nc.vector.tensor_tensor(out=xc[:], in0=xc[:], in1=wslice("g_rep", i),
                            op=mybir.AluOpType.mult)
    nc.scalar.mul(xc[:], xc[:], rstd[:, 0:1])  # ACT: xc *= rstd
    nc.vector.tensor_tensor(out=xc[:], in0=xc[:], in1=wslice("bln_rep", i),
                            op=mybir.AluOpType.add)
    relu = ep.tile([128, H], F32, tag="relu")
    nc.scalar.activation(relu[:], xc[:], mybir.ActivationFunctionType.Relu,
                         bias=zero_col[:, 0:1])
    if i == 2:
        res_r = ep.tile([128, H], F32, tag="resr")
        nc.sync.dma_start(res_r[:], res_dram[ns, :])
        nc.vector.tensor_tensor(out=h_sb[:, ns], in0=relu[:], in1=res_r[:],
                                op=mybir.AluOpType.add)
    else:
        nc.vector.tensor_tensor(out=h_sb[:, ns], in0=relu[:], in1=h_sb[:, ns],
                                op=mybir.AluOpType.add)




# ======================= axon NTFF profiling shim =======================

import contextlib
import ctypes
import sys
import types

_SO_PATH = "/opt/axon/libaxon_pjrt.so"


def _ntff_profile_via_ctypes(so_path: str):
    lib = ctypes.CDLL(so_path)
    if not hasattr(lib, "axon_start_nrt_profile"):
        return None
    lib.axon_start_nrt_profile.argtypes = [
        ctypes.POINTER(ctypes.c_int64),
        ctypes.c_size_t,
    ]
    lib.axon_start_nrt_profile.restype = ctypes.c_int64
    lib.axon_stop_nrt_profile.argtypes = [ctypes.c_char_p]
    lib.axon_stop_nrt_profile.restype = ctypes.c_int64

    @contextlib.contextmanager
    def _hook(output_dir: str, device_ids):
        import jax

        jax.devices()
        if device_ids:
            ids = (ctypes.c_int64 * len(device_ids))(*device_ids)
            rc = lib.axon_start_nrt_profile(ids, len(device_ids))
        else:
            rc = lib.axon_start_nrt_profile(None, 0)
        if rc != 0:
            raise RuntimeError(f"axon_start_nrt_profile rc={rc}")
        try:
            yield
        finally:
            n = lib.axon_stop_nrt_profile(str(output_dir).encode())
            if n < 0:
                raise RuntimeError(f"axon_stop_nrt_profile rc={n}")
            print(f"profile: {n} file(s) written to {output_dir}", file=sys.stderr)

    return _hook


def _shim_install():
    try:
        import antenv.axon_hooks  # noqa: F401

        return  # already present
    except ImportError:
        pass
    import antenv

    hook = _ntff_profile_via_ctypes(_SO_PATH)
    mod = types.ModuleType("antenv.axon_hooks")
    mod._hook = hook

    def get_axon_ntff_profile_hook():
        return mod._hook

    def set_axon_ntff_profile_hook(h):
        mod._hook = h

    mod.get_axon_ntff_profile_hook = get_axon_ntff_profile_hook
    mod.set_axon_ntff_profile_hook = set_axon_ntff_profile_hook
    sys.modules["antenv.axon_hooks"] = mod
    antenv.axon_hooks = mod


# ======================= kernel entry point =======================
_CACHE = {}
TRACE = False
LAST_EXEC_NS = None


def kernel(**inputs):
    """Full unsharded inputs -> full output [1, 200] (float32).

    Shards the graph across 8 NeuronCores internally (node partitioning with
    per-core degree relabeling; AllGather exchange of the normalized conv
    features per layer; per-core gather/scatter message passing; AllReduce
    mean-pool; replicated head).
    """
    global LAST_EXEC_NS
    from concourse import bass_utils

    cfg = Cfg()
    x = np.asarray(inputs["x"], np.float32)
    edge_index = np.asarray(inputs["edge_index"])
    assert x.shape == (cfg.N, cfg.F_IN) and edge_index.shape == (2, cfg.E)

    pc, meta = host_preprocess(cfg, x, edge_index)
    wd = prep_weights(cfg, inputs)

    key = "prog"
    if key not in _CACHE:
        _CACHE[key] = build_program(cfg, meta)
    nc = _CACHE[key]

    in_maps = []
    for c in range(cfg.NC):
        m = {"dinv": pc["dinv_dev"][c], "xT": np.ascontiguousarray(pc["xT"][c])}
        for g in range(meta.G):
            m[f"srcs{g}"] = pc["srcs_dev"][g][c]
            m[f"nrel{g}"] = pc["nrel_dev"][g][c]
        m.update(wd)
        in_maps.append(m)

    trace = TRACE
    if trace:
        try:
            _shim_install()
        except Exception:
            trace = False
    try:
        res = bass_utils.run_bass_kernel_spmd(
            nc, in_maps, core_ids=list(range(cfg.NC)), trace=trace)
    except Exception:
        if not trace:
            raise
        res = bass_utils.run_bass_kernel_spmd(
            nc, in_maps, core_ids=list(range(cfg.NC)), trace=False)
    LAST_EXEC_NS = res.exec_time_ns
    return np.asarray(res.results[0]["y"], np.float32)

